# revision 24
# baseline (speedup 1.0000x reference)
"""Multi-head attention (B=2, F=T=2048, H=1024, 16 heads x 64) on 8 TRN2
NeuronCores.

v3 design:
  * exp split across engines: per attention step, head j=0's exp runs on
    ACT (exact, free affine scale), head j=1's exp runs on DVE as a
    one-instruction Schraudolph bf16 exp (tensor_scalar mult+add with
    int16 output whose bits ARE the bf16 exp).
  * softmax normalization deferred to AFTER the AllToAll: shards carry 2
    denominator rows; denominators are inverted post-A2A with
    reciprocal_approx_fast on 16 partitions and broadcast via a tiny
    bf16 ones-matmul, folded into the bf16 conversion multiply.
  * AllToAll split per batch; A2A(b0) overlaps b1's attention. Output
    resharding: core c produces f-rows [256c, 256c+256) of BOTH batches.
  * work interleaving: batch-1 projections are emitted as side-closures
    inside batch-0's attention step loop (keeps PE dense/warm and the
    exp engines fed); batch-0's output projection is interleaved into
    batch-1's attention. Only b1's A2A + output projection are exposed.
  * inputs arrive chunk-major ([B*4, 128, HT*512]) so every DMA line is
    8KB contiguous.
"""

from contextlib import ExitStack

import ml_dtypes
import numpy as np

import concourse.bass as bass  # noqa: F401
import concourse.mybir as mybir
import concourse.tile as tile
from concourse import bacc
from concourse.bass_utils import run_bass_kernel_spmd

B, F, T, HID, NH, DH = 2, 2048, 2048, 1024, 16, 64
HT = HID // 128  # 8 h-tiles
TT = T // 128  # 16 key tiles
FC = F // 512  # 4 query chunks per batch
FS = 256  # f-rows per core per batch
SEG = 130 * 256  # A2A shard: 128 A^T rows + 2 denom rows, 256 f-cols
BF16, F32 = mybir.dt.bfloat16, mybir.dt.float32
I16 = mybir.dt.int16
NPBF16 = ml_dtypes.bfloat16

# Schraudolph bf16 exp: bits(int16) = s * ASCHR + BSCHR, s = raw logits.
ASCHR = 16.0 / float(np.log(2.0))  # (128/ln2) * (1/8 logit scale)
BSCHR = 16250.375  # calibrated; rel err +-3.3%, robust to trunc/nearest

OB_CONST = np.tile(np.kron(np.eye(2), np.ones((1, 64))), (8, 1)).astype(NPBF16)

_CACHE: dict = {}


def _build():
    nc = bacc.Bacc("TRN2", target_bir_lowering=False, debug=False, num_devices=8)

    qT = nc.declare_dram_parameter("qT", [B * 4, 128, HT * 512], BF16, isOutput=False)
    sT = nc.declare_dram_parameter("sT", [B * 4, 128, HT * 512], BF16, isOutput=False)
    w3 = nc.declare_dram_parameter("w3", [128, 3072], BF16, isOutput=False)
    wo = nc.declare_dram_parameter("wo", [HID, HID], BF16, isOutput=False)
    ob = nc.declare_dram_parameter("ob", [16, 128], BF16, isOutput=False)
    out = nc.declare_dram_parameter("out", [2 * FS, HID], F32, isOutput=True)

    a2a_in = [nc.dram_tensor(f"a2a_in{b}", [8, SEG], BF16) for b in range(B)]
    a2a_out = [nc.dram_tensor(f"a2a_out{b}", [8, SEG], BF16) for b in range(B)]

    with tile.TileContext(nc) as tc, ExitStack() as ctx:
        persist = ctx.enter_context(tc.tile_pool(name="persist", bufs=1))
        kT_sb = persist.tile([128, B, T], BF16, tag="kT")
        v_sb = persist.tile([128, B, TT, 2, DH + 1], BF16, tag="v")
        qTp_sb = persist.tile([128, B, F], BF16, tag="qTp")
        wo_sb = persist.tile([128, HT, HID], BF16, tag="wo")
        w3_sb = persist.tile([128, HT, 3, 128], BF16, tag="w3")  # wq|wk|wv
        onesb = persist.tile([16, 128], BF16, tag="onesb")

        nc.vector.memset(v_sb[:, :, :, :, DH : DH + 1], 1.0)
        nc.scalar.dma_start(out=onesb[:, :], in_=ob[:, :])
        nc.scalar.dma_start(
            out=w3_sb[:, :, :, :],
            in_=w3[:, :].rearrange("p (a k n) -> p a k n", a=HT, k=3),
        )

        with (
            tc.tile_pool(name="qin", bufs=4) as qin_pool,
            tc.tile_pool(name="sin", bufs=4) as sin_pool,
            tc.tile_pool(name="ptp", bufs=6) as pt_pool,
            tc.tile_pool(name="stg", bufs=3) as stg_pool,
            tc.tile_pool(name="den", bufs=2) as den_pool,
            tc.tile_pool(name="atg", bufs=2) as atg_pool,
            tc.tile_pool(name="stp", bufs=12) as st_pool,
            tc.tile_pool(name="otp", bufs=4) as ot_pool,
            tc.tile_pool(name="s_ps", bufs=2, space="PSUM") as s_ps_pool,
            tc.tile_pool(name="a_ps", bufs=1, space="PSUM") as a_ps_pool,
            tc.tile_pool(name="pj_ps", bufs=2, space="PSUM") as pj_ps,
        ):

            q_t = {0: [None] * 4, 1: [None] * 4}
            s_t = {0: [None] * 4, 1: [None] * 4}

            sc0_halves = []

            def dma_in_s00():
                for h in range(2):
                    sh = sin_pool.tile([128, 4, 512], BF16, tag="sc", name="sc0h")
                    nc.sync.dma_start(
                        out=sh[:, :, :],
                        in_=sT[0, :, 2048 * h : 2048 * (h + 1)].rearrange(
                            "p (a n) -> p a n", a=4
                        ),
                    )
                    sc0_halves.append(sh)

            def dma_in(kind, b, c):
                if kind == "q":
                    qc = qin_pool.tile([128, HT, 512], BF16, tag="qc")
                    nc.sync.dma_start(
                        out=qc[:, :, :],
                        in_=qT[4 * b + c, :, :].rearrange("p (a n) -> p a n", a=HT),
                    )
                    q_t[b][c] = qc
                else:
                    sc = sin_pool.tile([128, HT, 512], BF16, tag="sc")
                    nc.sync.dma_start(
                        out=sc[:, :, :],
                        in_=sT[4 * b + c, :, :].rearrange("p (a n) -> p a n", a=HT),
                    )
                    s_t[b][c] = sc

            def proj_closures(b, qs, ss, use_scalar_copies):
                """Per-chunk projection closures (each emits PE matmuls and
                one PSUM->SBUF copy)."""
                cls = []

                def qk_copy(dst, src):
                    if use_scalar_copies:
                        nc.scalar.copy(out=dst, in_=src)
                    else:
                        nc.vector.tensor_copy(out=dst, in_=src)

                def src_s(c, ht):
                    if b == 0 and c == 0:
                        return sc0_halves[ht // 4][:, ht % 4, :]
                    return ss[c][:, ht, :]

                def mk_k(c):
                    def f():
                        ps = pj_ps.tile([128, 512], F32, tag="pj", name="ps_k")
                        for ht in range(HT):
                            nc.tensor.matmul(
                                ps[:, :],
                                lhsT=w3_sb[:, ht, 1, :],
                                rhs=src_s(c, ht),
                                start=(ht == 0),
                                stop=(ht == HT - 1),
                            )
                        qk_copy(kT_sb[:, b, 512 * c : 512 * (c + 1)], ps[:, :])
                    return f

                def mk_v(c, i):
                    def f():
                        tt = 4 * c + i
                        ps = pj_ps.tile([128, 512], F32, tag="pj", name="ps_v")
                        for ht in range(HT):
                            nc.tensor.matmul(
                                ps[:, 0:128],
                                lhsT=src_s(c, ht)[:, 128 * i : 128 * (i + 1)],
                                rhs=w3_sb[:, ht, 2, :],
                                start=(ht == 0),
                                stop=(ht == HT - 1),
                            )
                        nc.vector.tensor_copy(
                            out=v_sb[:, b, tt, :, 0:DH],
                            in_=ps[:, 0:128].rearrange("p (j d) -> p j d", j=2),
                        )
                    return f

                def mk_q(c):
                    def f():
                        ps = pj_ps.tile([128, 512], F32, tag="pj", name="ps_q")
                        for ht in range(HT):
                            nc.tensor.matmul(
                                ps[:, :],
                                lhsT=w3_sb[:, ht, 0, :],
                                rhs=qs[c][:, ht, :],
                                start=(ht == 0),
                                stop=(ht == HT - 1),
                            )
                        qk_copy(qTp_sb[:, b, 512 * c : 512 * (c + 1)], ps[:, :])
                    return f

                for c in range(4):
                    cls.append(mk_k(c))
                    for i in range(4):
                        cls.append(mk_v(c, i))
                    cls.append(mk_q(c))
                return cls  # 24 closures (6 per chunk), PE-ordered K,V...,Q

            def emit_s_exp(b, fc, tt):
                # per-head S tiles and pt tiles: dependency tracking is
                # tile-granular, so splitting lets exp(j0) start right after
                # S(j0) and P@V(j0) wait only on its own exp engine.
                pts = []
                for j in range(2):
                    sp = s_ps_pool.tile([128, 512], F32, tag=f"s{j}", name="sp")
                    nc.tensor.matmul(
                        sp[:, :],
                        lhsT=kT_sb[
                            64 * j : 64 * (j + 1), b, 128 * tt : 128 * (tt + 1)
                        ],
                        rhs=qTp_sb[
                            64 * j : 64 * (j + 1), b, 512 * fc : 512 * (fc + 1)
                        ],
                        start=True,
                        stop=True,
                        tile_position=(64 * j, 0),
                    )
                    pt = pt_pool.tile([128, 512], BF16, tag=f"pt{j}", name="pt")
                    if j == 0:
                        nc.scalar.activation(
                            out=pt[:, :],
                            in_=sp[:, :],
                            func=mybir.ActivationFunctionType.Exp,
                            scale=float(DH) ** -0.5,
                        )
                    else:
                        with nc.allow_low_precision("schraudolph bf16 exp"):
                            nc.vector.tensor_scalar(
                                out=pt[:, :].bitcast(I16),
                                in0=sp[:, :],
                                scalar1=ASCHR,
                                scalar2=BSCHR,
                                op0=mybir.AluOpType.mult,
                                op1=mybir.AluOpType.add,
                            )
                    pts.append(pt)
                return pts

            def flush(b, fc, a_ps):
                stf = stg_pool.tile([65, 2, 512], BF16, tag="stf")
                with nc.allow_low_precision("bf16 a2a shard"):
                    nc.vector.tensor_copy(
                        out=stf[:, 0, :], in_=a_ps[:, 0, :]
                    )
                    nc.scalar.copy(out=stf[:, 1, :], in_=a_ps[:, 1, :])
                for h in range(2):
                    dst = 2 * fc + h
                    sl = np.s_[256 * h : 256 * (h + 1)]
                    nc.scalar.dma_start(
                        out=a2a_in[b][dst, 0:32768].rearrange(
                            "(j p n) -> p j n", j=2, p=64, n=256
                        ),
                        in_=stf[0:64, :, sl],
                    )
                    nc.scalar.dma_start(
                        out=a2a_in[b][dst, 32768:33280].rearrange(
                            "(o j n) -> o j n", o=1, j=2, n=256
                        ),
                        in_=stf[64:65, :, sl],
                    )

            def outproj_closures(b):
                """Post-A2A(b): reciprocal + broadcast + normalize-multiply +
                output projection, as closures."""
                cls = []
                den2b = den_pool.tile([2, 8, 256], BF16, tag="den2b")
                den2f = den_pool.tile([2, 8, 256], F32, tag="den2f")
                den2r = den_pool.tile([2, 8, 256], F32, tag="den2r")
                den2 = den_pool.tile([2, 8, 256], BF16, tag="den2")
                atg = atg_pool.tile([128, 8, 256], BF16, tag="atg")

                def gather():
                    nc.sync.dma_start(
                        out=den2b[:, :, :],
                        in_=a2a_out[b][:, 32768:33280].rearrange(
                            "a (j n) -> j a n", j=2, n=256
                        ),
                    )
                    nc.scalar.dma_start(
                        out=atg[:, :, :],
                        in_=a2a_out[b][:, 0:32768].rearrange(
                            "a (r n) -> r a n", r=128, n=256
                        ),
                    )

                def recip():
                    nc.vector.tensor_copy(out=den2f[:, :, :], in_=den2b[:, :, :])
                    nc.vector.reciprocal_approx_fast(
                        out=den2r[:, :, :], in_=den2f[:, :, :]
                    )
                    with nc.allow_low_precision("bf16 denom recip"):
                        nc.vector.tensor_copy(out=den2[:, :, :], in_=den2r[:, :, :])

                cls.append(gather)
                cls.append(recip)
                sts = [None] * 8

                def mk_bc(a):
                    def f():
                        bc = pj_ps.tile([128, 512], F32, tag="pj", name="bc")
                        nc.tensor.matmul(
                            bc[:, 0:256],
                            lhsT=onesb[0:2, :],
                            rhs=den2[:, a, :],
                            start=True,
                            stop=True,
                        )
                        st = st_pool.tile([128, 256], BF16, tag="st")
                        with nc.allow_low_precision("bf16 attn normalize"):
                            nc.vector.tensor_mul(
                                out=st[:, :], in0=atg[:, a, :], in1=bc[:, 0:256]
                            )
                        sts[a] = st
                    return f

                for a in range(8):
                    cls.append(mk_bc(a))

                def mk_o(ft, jj):
                    def f():
                        o_ps = pj_ps.tile([128, 512], F32, tag="pj", name="o")
                        for a in range(8):
                            nc.tensor.matmul(
                                o_ps[:, :],
                                lhsT=sts[a][:, 128 * ft : 128 * (ft + 1)],
                                rhs=wo_sb[:, a, 512 * jj : 512 * (jj + 1)],
                                start=(a == 0),
                                stop=(a == 7),
                            )
                        ot = ot_pool.tile([128, 512], F32, tag="ot")
                        if jj == 0:
                            nc.vector.tensor_copy(out=ot[:, :], in_=o_ps[:, :])
                        else:
                            nc.scalar.copy(out=ot[:, :], in_=o_ps[:, :])
                        nc.scalar.dma_start(
                            out=out[
                                256 * b + 128 * ft : 256 * b + 128 * (ft + 1),
                                512 * jj : 512 * (jj + 1),
                            ],
                            in_=ot[:, :],
                        )
                    return f

                for ft in range(2):
                    for jj in range(2):
                        cls.append(mk_o(ft, jj))
                return cls  # 14 closures

            def attention_batch(b, sched):
                """sched: dict step -> list of closures to emit at that step."""
                steps = [(fc, tt) for fc in range(FC) for tt in range(TT)]
                pts = {}
                pts[steps[0]] = emit_s_exp(b, *steps[0])
                a_tiles = {}
                for i, (fc, tt) in enumerate(steps):
                    if tt == 0:
                        a_tiles[fc] = a_ps_pool.tile(
                            [65, 2, 512], F32, tag="a", name="a_acc"
                        )
                    last = tt == TT - 1
                    if not last and i + 1 < len(steps):
                        pts[steps[i + 1]] = emit_s_exp(b, *steps[i + 1])
                    # side work lands between the S matmuls and the P@V so the
                    # PE stays busy while this step's exp finishes
                    for cl in sched.get(i, ()):
                        cl()
                    a_ps = a_tiles[fc]
                    pt = pts.pop((fc, tt))
                    for j in (1, 0):
                        nc.tensor.matmul(
                            a_ps[:, j, :],
                            lhsT=v_sb[:, b, tt, j, :],
                            rhs=pt[j][:, :],
                            start=(tt == 0),
                            stop=(tt == TT - 1),
                        )
                    if last:
                        # flush first so the a_ps staging copy leads the exp
                        # engines' queues; next fc's first PV then has its
                        # accumulator free with no stall.
                        flush(b, fc, a_ps)
                        del a_tiles[fc]
                        if i + 1 < len(steps):
                            pts[steps[i + 1]] = emit_s_exp(b, *steps[i + 1])
                for i in sorted(k for k in sched if k >= len(steps)):
                    for cl in sched[i]:
                        cl()

            def spread(closures, start, end):
                """Schedule closures evenly over steps [start, end)."""
                sched = {}
                n = len(closures)
                for idx, cl in enumerate(closures):
                    step = start + idx * (end - start) // n
                    sched.setdefault(step, []).append(cl)
                return sched

            # ---- phase structure ------------------------------------------
            # Input DMAs in consumption-priority order (sync queue is FIFO).
            dma_in_s00()
            dma_in("q", 0, 0)
            for c in range(1, 4):
                dma_in("s", 0, c)
            dma_in("q", 0, 1)
            dma_in("s", 1, 0)
            dma_in("q", 0, 2)
            dma_in("s", 1, 1)
            dma_in("q", 0, 3)
            dma_in("s", 1, 2)
            dma_in("s", 1, 3)
            for c in range(4):
                dma_in("q", 1, c)

            p0 = proj_closures(0, q_t[0], s_t[0], use_scalar_copies=False)
            # chunk 0 of batch 0 runs serially (ramp): K, V x4, Q
            for cl in p0[0:6]:
                cl()
            p1 = proj_closures(1, q_t[1], s_t[1], use_scalar_copies=True)

            # b0 attention schedule: K-c by step 4(c-1), V-c(tt) by step tt,
            # Q-c late (needed at step 16c); b1 K/V spread mid, Q last.
            sched0 = {}
            for c in range(1, 4):
                k_cl, v_cls, q_cl = p0[6 * c], p0[6 * c + 1 : 6 * c + 5], p0[6 * c + 5]
                sched0.setdefault(4 * (c - 1), []).append(k_cl)
                for i, cl in enumerate(v_cls):
                    sched0.setdefault(4 * (c - 1) + 1 + (3 * i) // 4, []).append(cl)
                sched0.setdefault(16 * c - 6, []).append(q_cl)
            b1_kv = []
            b1_q = []
            for c in range(4):
                b1_kv.append(p1[6 * c])
                b1_kv.extend(p1[6 * c + 1 : 6 * c + 5])
                b1_q.append(p1[6 * c + 5])
            for step, cls in spread(b1_kv, 16, 56).items():
                sched0.setdefault(step, []).extend(cls)
            for step, cls in spread(b1_q, 56, 64).items():
                sched0.setdefault(step, []).extend(cls)
            attention_batch(0, sched0)

            nc.gpsimd.collective_compute(
                "AllToAll",
                mybir.AluOpType.bypass,
                replica_groups=[[0, 1, 2, 3, 4, 5, 6, 7]],
                ins=[a2a_in[0].ap().opt()],
                outs=[a2a_out[0].ap().opt()],
            )

            # wo only needed from b0's output projection onward.
            nc.scalar.dma_start(
                out=wo_sb[:, :, :],
                in_=wo[:, :].rearrange("(a p) n -> p a n", p=128),
            )

            # b1 attention with b0's output projection interleaved late;
            # b0's final matmul groups run after b1's attention, inside the
            # A2A#2 wait window.
            op0 = outproj_closures(0)
            sched1 = spread(op0[:10], 18, 54)
            sched1.setdefault(58, []).append(op0[10])
            sched1.setdefault(61, []).append(op0[11])
            sched1[64] = op0[12:]
            attention_batch(1, sched1)

            nc.gpsimd.collective_compute(
                "AllToAll",
                mybir.AluOpType.bypass,
                replica_groups=[[0, 1, 2, 3, 4, 5, 6, 7]],
                ins=[a2a_in[1].ap().opt()],
                outs=[a2a_out[1].ap().opt()],
            )

            for cl in outproj_closures(1):
                cl()

    nc.compile()
    return nc


def _get_nc():
    if "nc" not in _CACHE:
        _CACHE["nc"] = _build()
    return _CACHE["nc"]


def _reference_fallback(query_input, source_input, bias, wq, wk, wv, wo):
    """Numpy fallback, only used if bias is unexpectedly nonzero."""
    q = np.einsum("bfh,hnd->bfnd", query_input, wq) * (DH**-0.5)
    k = np.einsum("bth,hnd->btnd", source_input, wk)
    v = np.einsum("bth,hnd->btnd", source_input, wv)
    logits = np.einsum("btnd,bfnd->bnft", k, q) + bias
    logits -= logits.max(axis=-1, keepdims=True)
    w = np.exp(logits)
    w /= w.sum(axis=-1, keepdims=True)
    attn = np.einsum("bnft,btnd->bfnd", w, v)
    return np.einsum("bfnd,ndh->bfh", attn, wo).astype(np.float32)


def _chunk_major(xT):
    """[B, HID, F] -> [B*4, 128, HT*512] with 8KB-contiguous lines:
    out[4b+c, p, a*512+n] = xT[b, a*128+p, 512c+n]."""
    o = np.empty((B * 4, 128, HT * 512), dtype=NPBF16)
    for b in range(B):
        for c in range(4):
            chunk = xT[b, :, 512 * c : 512 * (c + 1)]  # [1024, 512]
            o[4 * b + c] = (
                chunk.reshape(HT, 128, 512).transpose(1, 0, 2).reshape(128, HT * 512)
            )
    return np.ascontiguousarray(o)


def make_in_maps(query_input, source_input, wq, wk, wv, wo):
    wo2 = np.ascontiguousarray(wo.reshape(HID, HID).astype(NPBF16))
    qTb = _chunk_major(np.transpose(query_input, (0, 2, 1)).astype(NPBF16))
    sTb = _chunk_major(np.transpose(source_input, (0, 2, 1)).astype(NPBF16))
    wqh = wq.reshape(HID, NH, DH)
    wkh = wk.reshape(HID, NH, DH)
    wvh = wv.reshape(HID, NH, DH)

    in_maps = []
    for c in range(8):
        sl = np.s_[:, 2 * c : 2 * c + 2, :]
        w3c = np.concatenate(
            [
                wqh[sl].reshape(HID, 128),
                wkh[sl].reshape(HID, 128),
                wvh[sl].reshape(HID, 128),
            ],
            axis=1,
        )  # [1024, 384]
        w3c = (
            w3c.reshape(HT, 128, 3, 128).transpose(1, 0, 2, 3).reshape(128, 3072)
        )
        in_maps.append(
            {
                "qT": qTb,
                "sT": sTb,
                "w3": np.ascontiguousarray(w3c).astype(NPBF16),
                "wo": wo2,
                "ob": OB_CONST,
            }
        )
    return in_maps


def assemble(results):
    """results[c]["out"] is [512, 1024]: rows 0-255 = batch 0 f-slice
    [256c, 256c+256), rows 256-511 = batch 1 same slice."""
    out_full = np.empty((B, F, HID), dtype=np.float32)
    for c in range(8):
        r = results[c]["out"]
        out_full[0, FS * c : FS * (c + 1), :] = r[0:FS]
        out_full[1, FS * c : FS * (c + 1), :] = r[FS : 2 * FS]
    return out_full


def kernel(query_input, source_input, bias, wq, wk, wv, wo):
    query_input = np.asarray(query_input, dtype=np.float32)
    source_input = np.asarray(source_input, dtype=np.float32)
    bias = np.asarray(bias, dtype=np.float32)
    wq = np.asarray(wq, dtype=np.float32)
    wk = np.asarray(wk, dtype=np.float32)
    wv = np.asarray(wv, dtype=np.float32)
    wo = np.asarray(wo, dtype=np.float32)

    if np.any(bias):
        return _reference_fallback(query_input, source_input, bias, wq, wk, wv, wo)

    in_maps = make_in_maps(query_input, source_input, wq, wk, wv, wo)
    nc = _get_nc()
    res = run_bass_kernel_spmd(nc, in_maps, core_ids=list(range(8)))
    return assemble(res.results)


# revision 25
# speedup vs baseline: 1.3172x; 1.3172x over previous
"""Multi-head attention (B=2, F=T=2048, H=1024, 16 heads x 64) on 8 TRN2
NeuronCores.

v3 design:
  * exp split across engines: per attention step, head j=0's exp runs on
    ACT (exact, free affine scale), head j=1's exp runs on DVE as a
    one-instruction Schraudolph bf16 exp (tensor_scalar mult+add with
    int16 output whose bits ARE the bf16 exp).
  * softmax normalization deferred to AFTER the AllToAll: shards carry 2
    denominator rows; denominators are inverted post-A2A with
    reciprocal_approx_fast on 16 partitions and broadcast via a tiny
    bf16 ones-matmul, folded into the bf16 conversion multiply.
  * AllToAll split per batch; A2A(b0) overlaps b1's attention. Output
    resharding: core c produces f-rows [256c, 256c+256) of BOTH batches.
  * work interleaving: batch-1 projections are emitted as side-closures
    inside batch-0's attention step loop (keeps PE dense/warm and the
    exp engines fed); batch-0's output projection is interleaved into
    batch-1's attention. Only b1's A2A + output projection are exposed.
  * inputs arrive chunk-major ([B*4, 128, HT*512]) so every DMA line is
    8KB contiguous.
"""

from contextlib import ExitStack

import ml_dtypes
import numpy as np

import concourse.bass as bass  # noqa: F401
import concourse.mybir as mybir
import concourse.tile as tile
from concourse import bacc
from concourse.bass_utils import run_bass_kernel_spmd

B, F, T, HID, NH, DH = 2, 2048, 2048, 1024, 16, 64
HT = HID // 128  # 8 h-tiles
TT = T // 128  # 16 key tiles
FC = F // 512  # 4 query chunks per batch
FS = 256  # f-rows per core per batch
SEG = 130 * 256  # A2A shard: 128 A^T rows + 2 denom rows, 256 f-cols
BF16, F32 = mybir.dt.bfloat16, mybir.dt.float32
I16 = mybir.dt.int16
NPBF16 = ml_dtypes.bfloat16

# Schraudolph bf16 exp: bits(int16) = s * ASCHR + BSCHR, s = raw logits.
ASCHR = 16.0 / float(np.log(2.0))  # (128/ln2) * (1/8 logit scale)
BSCHR = 16250.375  # calibrated; rel err +-3.3%, robust to trunc/nearest

OB_CONST = np.tile(np.kron(np.eye(2), np.ones((1, 64))), (8, 1)).astype(NPBF16)

_CACHE: dict = {}


def _build():
    nc = bacc.Bacc("TRN2", target_bir_lowering=False, debug=False, num_devices=8)

    qT = nc.declare_dram_parameter("qT", [B * 4, 128, HT * 512], BF16, isOutput=False)
    sT = nc.declare_dram_parameter("sT", [B * 4, 128, HT * 512], BF16, isOutput=False)
    w3 = nc.declare_dram_parameter("w3", [128, 3072], BF16, isOutput=False)
    wo = nc.declare_dram_parameter("wo", [HID, HID], BF16, isOutput=False)
    ob = nc.declare_dram_parameter("ob", [16, 128], BF16, isOutput=False)
    out = nc.declare_dram_parameter("out", [2 * FS, HID], F32, isOutput=True)

    a2a_in = [nc.dram_tensor(f"a2a_in{b}", [8, SEG], BF16) for b in range(B)]
    a2a_out = [nc.dram_tensor(f"a2a_out{b}", [8, SEG], BF16) for b in range(B)]

    with tile.TileContext(nc) as tc, ExitStack() as ctx:
        persist = ctx.enter_context(tc.tile_pool(name="persist", bufs=1))
        kT_sb = persist.tile([128, B, T], BF16, tag="kT")
        v_sb = persist.tile([128, B, TT, 2, DH + 1], BF16, tag="v")
        qTp_sb = persist.tile([128, B, F], BF16, tag="qTp")
        wo_sb = persist.tile([128, HT, HID], BF16, tag="wo")
        w3_sb = persist.tile([128, 3, HT, 128], BF16, tag="w3")  # k-major
        onesb = persist.tile([16, 128], BF16, tag="onesb")

        nc.vector.memset(v_sb[:, :, :, :, DH : DH + 1], 1.0)
        nc.scalar.dma_start(out=onesb[:, :], in_=ob[:, :])
        for k in (1, 2, 0):  # K first (gates the ramp), then V, then Q
            nc.scalar.dma_start(
                out=w3_sb[:, k, :, :],
                in_=w3[:, 1024 * k : 1024 * (k + 1)].rearrange(
                    "p (a n) -> p a n", a=HT
                ),
            )

        with (
            tc.tile_pool(name="qin", bufs=4) as qin_pool,
            tc.tile_pool(name="sin", bufs=4) as sin_pool,
            tc.tile_pool(name="ptp", bufs=6) as pt_pool,
            tc.tile_pool(name="stg", bufs=3) as stg_pool,
            tc.tile_pool(name="den", bufs=2) as den_pool,
            tc.tile_pool(name="atg", bufs=2) as atg_pool,
            tc.tile_pool(name="stp", bufs=12) as st_pool,
            tc.tile_pool(name="otp", bufs=4) as ot_pool,
            tc.tile_pool(name="s_ps", bufs=2, space="PSUM") as s_ps_pool,
            tc.tile_pool(name="a_ps", bufs=1, space="PSUM") as a_ps_pool,
            tc.tile_pool(name="pj_ps", bufs=2, space="PSUM") as pj_ps,
        ):

            q_t = {0: [None] * 4, 1: [None] * 4}
            s_t = {0: [None] * 4, 1: [None] * 4}

            sc0_halves = []

            def dma_in_s00():
                for h in range(2):
                    sh = sin_pool.tile([128, 4, 512], BF16, tag="sc", name="sc0h")
                    nc.sync.dma_start(
                        out=sh[:, :, :],
                        in_=sT[0, :, 2048 * h : 2048 * (h + 1)].rearrange(
                            "p (a n) -> p a n", a=4
                        ),
                    )
                    sc0_halves.append(sh)

            def dma_in(kind, b, c):
                if kind == "q":
                    qc = qin_pool.tile([128, HT, 512], BF16, tag="qc")
                    nc.sync.dma_start(
                        out=qc[:, :, :],
                        in_=qT[4 * b + c, :, :].rearrange("p (a n) -> p a n", a=HT),
                    )
                    q_t[b][c] = qc
                else:
                    sc = sin_pool.tile([128, HT, 512], BF16, tag="sc")
                    nc.sync.dma_start(
                        out=sc[:, :, :],
                        in_=sT[4 * b + c, :, :].rearrange("p (a n) -> p a n", a=HT),
                    )
                    s_t[b][c] = sc

            def proj_closures(b, qs, ss, use_scalar_copies):
                """Per-chunk projection closures (each emits PE matmuls and
                one PSUM->SBUF copy)."""
                cls = []

                def qk_copy(dst, src):
                    if use_scalar_copies:
                        nc.scalar.copy(out=dst, in_=src)
                    else:
                        nc.vector.tensor_copy(out=dst, in_=src)

                def src_s(c, ht):
                    if b == 0 and c == 0:
                        return sc0_halves[ht // 4][:, ht % 4, :]
                    return ss[c][:, ht, :]

                def mk_k(c):
                    def f():
                        ps = pj_ps.tile([128, 512], F32, tag="pj", name="ps_k")
                        for ht in range(HT):
                            nc.tensor.matmul(
                                ps[:, :],
                                lhsT=w3_sb[:, 1, ht, :],
                                rhs=src_s(c, ht),
                                start=(ht == 0),
                                stop=(ht == HT - 1),
                            )
                        qk_copy(kT_sb[:, b, 512 * c : 512 * (c + 1)], ps[:, :])
                    return f

                def mk_v(c, i):
                    def f():
                        tt = 4 * c + i
                        ps = pj_ps.tile([128, 512], F32, tag="pj", name="ps_v")
                        for ht in range(HT):
                            nc.tensor.matmul(
                                ps[:, 0:128],
                                lhsT=src_s(c, ht)[:, 128 * i : 128 * (i + 1)],
                                rhs=w3_sb[:, 2, ht, :],
                                start=(ht == 0),
                                stop=(ht == HT - 1),
                            )
                        nc.vector.tensor_copy(
                            out=v_sb[:, b, tt, :, 0:DH],
                            in_=ps[:, 0:128].rearrange("p (j d) -> p j d", j=2),
                        )
                    return f

                def mk_q(c):
                    def f():
                        ps = pj_ps.tile([128, 512], F32, tag="pj", name="ps_q")
                        for ht in range(HT):
                            nc.tensor.matmul(
                                ps[:, :],
                                lhsT=w3_sb[:, 0, ht, :],
                                rhs=qs[c][:, ht, :],
                                start=(ht == 0),
                                stop=(ht == HT - 1),
                            )
                        qk_copy(qTp_sb[:, b, 512 * c : 512 * (c + 1)], ps[:, :])
                    return f

                for c in range(4):
                    cls.append(mk_k(c))
                    for i in range(4):
                        cls.append(mk_v(c, i))
                    cls.append(mk_q(c))
                return cls  # 24 closures (6 per chunk), PE-ordered K,V...,Q

            def emit_s_exp(b, fc, tt):
                # per-head S tiles and pt tiles: dependency tracking is
                # tile-granular, so splitting lets exp(j0) start right after
                # S(j0) and P@V(j0) wait only on its own exp engine.
                pts = []
                for j in range(2):
                    sp = s_ps_pool.tile([128, 512], F32, tag=f"s{j}", name="sp")
                    nc.tensor.matmul(
                        sp[:, :],
                        lhsT=kT_sb[
                            64 * j : 64 * (j + 1), b, 128 * tt : 128 * (tt + 1)
                        ],
                        rhs=qTp_sb[
                            64 * j : 64 * (j + 1), b, 512 * fc : 512 * (fc + 1)
                        ],
                        start=True,
                        stop=True,
                        tile_position=(64 * j, 0),
                    )
                    pt = pt_pool.tile([128, 512], BF16, tag=f"pt{j}", name="pt")
                    if j == 0:
                        nc.scalar.activation(
                            out=pt[:, :],
                            in_=sp[:, :],
                            func=mybir.ActivationFunctionType.Exp,
                            scale=float(DH) ** -0.5,
                        )
                    else:
                        with nc.allow_low_precision("schraudolph bf16 exp"):
                            nc.vector.tensor_scalar(
                                out=pt[:, :].bitcast(I16),
                                in0=sp[:, :],
                                scalar1=ASCHR,
                                scalar2=BSCHR,
                                op0=mybir.AluOpType.mult,
                                op1=mybir.AluOpType.add,
                            )
                    pts.append(pt)
                return pts

            def flush(b, fc, a_ps):
                stf = stg_pool.tile([65, 2, 512], BF16, tag="stf")
                with nc.allow_low_precision("bf16 a2a shard"):
                    nc.vector.tensor_copy(
                        out=stf[:, 0, :], in_=a_ps[:, 0, :]
                    )
                    nc.scalar.copy(out=stf[:, 1, :], in_=a_ps[:, 1, :])
                for h in range(2):
                    dst = 2 * fc + h
                    sl = np.s_[256 * h : 256 * (h + 1)]
                    nc.scalar.dma_start(
                        out=a2a_in[b][dst, 0:32768].rearrange(
                            "(j p n) -> p j n", j=2, p=64, n=256
                        ),
                        in_=stf[0:64, :, sl],
                    )
                    nc.scalar.dma_start(
                        out=a2a_in[b][dst, 32768:33280].rearrange(
                            "(o j n) -> o j n", o=1, j=2, n=256
                        ),
                        in_=stf[64:65, :, sl],
                    )

            def outproj_closures(b):
                """Post-A2A(b): reciprocal + broadcast + normalize-multiply +
                output projection, as closures."""
                cls = []
                den2b = den_pool.tile([2, 8, 256], BF16, tag="den2b")
                den2f = den_pool.tile([2, 8, 256], F32, tag="den2f")
                den2r = den_pool.tile([2, 8, 256], F32, tag="den2r")
                den2 = den_pool.tile([2, 8, 256], BF16, tag="den2")
                atg = atg_pool.tile([128, 8, 256], BF16, tag="atg")

                def gather():
                    nc.sync.dma_start(
                        out=den2b[:, :, :],
                        in_=a2a_out[b][:, 32768:33280].rearrange(
                            "a (j n) -> j a n", j=2, n=256
                        ),
                    )
                    nc.scalar.dma_start(
                        out=atg[:, :, :],
                        in_=a2a_out[b][:, 0:32768].rearrange(
                            "a (r n) -> r a n", r=128, n=256
                        ),
                    )

                def recip():
                    nc.vector.tensor_copy(out=den2f[:, :, :], in_=den2b[:, :, :])
                    nc.vector.reciprocal_approx_fast(
                        out=den2r[:, :, :], in_=den2f[:, :, :]
                    )
                    with nc.allow_low_precision("bf16 denom recip"):
                        nc.vector.tensor_copy(out=den2[:, :, :], in_=den2r[:, :, :])

                cls.append(gather)
                cls.append(recip)
                sts = [None] * 8

                def mk_bc(a):
                    def f():
                        bc = pj_ps.tile([128, 512], F32, tag="pj", name="bc")
                        nc.tensor.matmul(
                            bc[:, 0:256],
                            lhsT=onesb[0:2, :],
                            rhs=den2[:, a, :],
                            start=True,
                            stop=True,
                        )
                        st = st_pool.tile([128, 256], BF16, tag="st")
                        with nc.allow_low_precision("bf16 attn normalize"):
                            nc.vector.tensor_mul(
                                out=st[:, :], in0=atg[:, a, :], in1=bc[:, 0:256]
                            )
                        sts[a] = st
                    return f

                for a in range(8):
                    cls.append(mk_bc(a))

                def mk_o(ft, jj):
                    def f():
                        o_ps = pj_ps.tile([128, 512], F32, tag="pj", name="o")
                        for a in range(8):
                            nc.tensor.matmul(
                                o_ps[:, :],
                                lhsT=sts[a][:, 128 * ft : 128 * (ft + 1)],
                                rhs=wo_sb[:, a, 512 * jj : 512 * (jj + 1)],
                                start=(a == 0),
                                stop=(a == 7),
                            )
                        ot = ot_pool.tile([128, 512], F32, tag="ot")
                        if jj == 0:
                            nc.vector.tensor_copy(out=ot[:, :], in_=o_ps[:, :])
                        else:
                            nc.scalar.copy(out=ot[:, :], in_=o_ps[:, :])
                        nc.scalar.dma_start(
                            out=out[
                                256 * b + 128 * ft : 256 * b + 128 * (ft + 1),
                                512 * jj : 512 * (jj + 1),
                            ],
                            in_=ot[:, :],
                        )
                    return f

                for ft in range(2):
                    for jj in range(2):
                        cls.append(mk_o(ft, jj))
                return cls  # 14 closures

            def attention_batch(b, sched):
                """sched: dict step -> list of closures to emit at that step."""
                steps = [(fc, tt) for fc in range(FC) for tt in range(TT)]
                pts = {}
                pts[steps[0]] = emit_s_exp(b, *steps[0])
                a_tiles = {}
                for i, (fc, tt) in enumerate(steps):
                    if tt == 0:
                        a_tiles[fc] = a_ps_pool.tile(
                            [65, 2, 512], F32, tag="a", name="a_acc"
                        )
                    last = tt == TT - 1
                    if not last and i + 1 < len(steps):
                        pts[steps[i + 1]] = emit_s_exp(b, *steps[i + 1])
                    # side work lands between the S matmuls and the P@V so the
                    # PE stays busy while this step's exp finishes
                    for cl in sched.get(i, ()):
                        cl()
                    a_ps = a_tiles[fc]
                    pt = pts.pop((fc, tt))
                    for j in (1, 0):
                        nc.tensor.matmul(
                            a_ps[:, j, :],
                            lhsT=v_sb[:, b, tt, j, :],
                            rhs=pt[j][:, :],
                            start=(tt == 0),
                            stop=(tt == TT - 1),
                        )
                    if last:
                        # flush first so the a_ps staging copy leads the exp
                        # engines' queues; next fc's first PV then has its
                        # accumulator free with no stall.
                        flush(b, fc, a_ps)
                        del a_tiles[fc]
                        if i + 1 < len(steps):
                            pts[steps[i + 1]] = emit_s_exp(b, *steps[i + 1])
                for i in sorted(k for k in sched if k >= len(steps)):
                    for cl in sched[i]:
                        cl()

            def spread(closures, start, end):
                """Schedule closures evenly over steps [start, end)."""
                sched = {}
                n = len(closures)
                for idx, cl in enumerate(closures):
                    step = start + idx * (end - start) // n
                    sched.setdefault(step, []).append(cl)
                return sched

            # ---- phase structure ------------------------------------------
            # Input DMAs in consumption-priority order (sync queue is FIFO).
            dma_in_s00()
            dma_in("q", 0, 0)
            for c in range(1, 4):
                dma_in("s", 0, c)
            dma_in("q", 0, 1)
            dma_in("s", 1, 0)
            dma_in("q", 0, 2)
            dma_in("s", 1, 1)
            dma_in("q", 0, 3)
            dma_in("s", 1, 2)
            dma_in("s", 1, 3)
            for c in range(4):
                dma_in("q", 1, c)

            p0 = proj_closures(0, q_t[0], s_t[0], use_scalar_copies=False)
            # chunk 0 of batch 0 runs serially (ramp): K, V x4, Q
            for cl in p0[0:6]:
                cl()
            p1 = proj_closures(1, q_t[1], s_t[1], use_scalar_copies=True)

            # b0 attention schedule: K-c by step 4(c-1), V-c(tt) by step tt,
            # Q-c late (needed at step 16c); b1 K/V spread mid, Q last.
            sched0 = {}
            for c in range(1, 4):
                k_cl, v_cls, q_cl = p0[6 * c], p0[6 * c + 1 : 6 * c + 5], p0[6 * c + 5]
                sched0.setdefault(4 * (c - 1), []).append(k_cl)
                for i, cl in enumerate(v_cls):
                    sched0.setdefault(4 * (c - 1) + 1 + (3 * i) // 4, []).append(cl)
                sched0.setdefault(16 * c - 6, []).append(q_cl)
            b1_kv = []
            b1_q = []
            for c in range(4):
                b1_kv.append(p1[6 * c])
                b1_kv.extend(p1[6 * c + 1 : 6 * c + 5])
                b1_q.append(p1[6 * c + 5])
            for step, cls in spread(b1_kv, 16, 56).items():
                sched0.setdefault(step, []).extend(cls)
            for step, cls in spread(b1_q, 56, 64).items():
                sched0.setdefault(step, []).extend(cls)
            attention_batch(0, sched0)

            nc.gpsimd.collective_compute(
                "AllToAll",
                mybir.AluOpType.bypass,
                replica_groups=[[0, 1, 2, 3, 4, 5, 6, 7]],
                ins=[a2a_in[0].ap().opt()],
                outs=[a2a_out[0].ap().opt()],
            )

            # wo only needed from b0's output projection onward.
            nc.scalar.dma_start(
                out=wo_sb[:, :, :],
                in_=wo[:, :].rearrange("(a p) n -> p a n", p=128),
            )

            # b1 attention with b0's output projection interleaved late;
            # b0's final matmul groups run after b1's attention, inside the
            # A2A#2 wait window.
            op0 = outproj_closures(0)
            sched1 = spread(op0[:10], 18, 54)
            sched1.setdefault(58, []).append(op0[10])
            sched1.setdefault(61, []).append(op0[11])
            sched1[64] = op0[12:]
            attention_batch(1, sched1)

            nc.gpsimd.collective_compute(
                "AllToAll",
                mybir.AluOpType.bypass,
                replica_groups=[[0, 1, 2, 3, 4, 5, 6, 7]],
                ins=[a2a_in[1].ap().opt()],
                outs=[a2a_out[1].ap().opt()],
            )

            for cl in outproj_closures(1):
                cl()

    nc.compile()
    return nc


def _get_nc():
    if "nc" not in _CACHE:
        _CACHE["nc"] = _build()
    return _CACHE["nc"]


def _reference_fallback(query_input, source_input, bias, wq, wk, wv, wo):
    """Numpy fallback, only used if bias is unexpectedly nonzero."""
    q = np.einsum("bfh,hnd->bfnd", query_input, wq) * (DH**-0.5)
    k = np.einsum("bth,hnd->btnd", source_input, wk)
    v = np.einsum("bth,hnd->btnd", source_input, wv)
    logits = np.einsum("btnd,bfnd->bnft", k, q) + bias
    logits -= logits.max(axis=-1, keepdims=True)
    w = np.exp(logits)
    w /= w.sum(axis=-1, keepdims=True)
    attn = np.einsum("bnft,btnd->bfnd", w, v)
    return np.einsum("bfnd,ndh->bfh", attn, wo).astype(np.float32)


def _chunk_major(xT):
    """[B, HID, F] -> [B*4, 128, HT*512] with 8KB-contiguous lines:
    out[4b+c, p, a*512+n] = xT[b, a*128+p, 512c+n]."""
    o = np.empty((B * 4, 128, HT * 512), dtype=NPBF16)
    for b in range(B):
        for c in range(4):
            chunk = xT[b, :, 512 * c : 512 * (c + 1)]  # [1024, 512]
            o[4 * b + c] = (
                chunk.reshape(HT, 128, 512).transpose(1, 0, 2).reshape(128, HT * 512)
            )
    return np.ascontiguousarray(o)


def make_in_maps(query_input, source_input, wq, wk, wv, wo):
    wo2 = np.ascontiguousarray(wo.reshape(HID, HID).astype(NPBF16))
    qTb = _chunk_major(np.transpose(query_input, (0, 2, 1)).astype(NPBF16))
    sTb = _chunk_major(np.transpose(source_input, (0, 2, 1)).astype(NPBF16))
    wqh = wq.reshape(HID, NH, DH)
    wkh = wk.reshape(HID, NH, DH)
    wvh = wv.reshape(HID, NH, DH)

    in_maps = []
    for c in range(8):
        sl = np.s_[:, 2 * c : 2 * c + 2, :]
        w3c = np.concatenate(
            [
                wqh[sl].reshape(HID, 128),
                wkh[sl].reshape(HID, 128),
                wvh[sl].reshape(HID, 128),
            ],
            axis=1,
        )  # [1024, 384]
        w3c = (
            w3c.reshape(HT, 128, 3, 128).transpose(1, 2, 0, 3).reshape(128, 3072)
        )
        in_maps.append(
            {
                "qT": qTb,
                "sT": sTb,
                "w3": np.ascontiguousarray(w3c).astype(NPBF16),
                "wo": wo2,
                "ob": OB_CONST,
            }
        )
    return in_maps


def assemble(results):
    """results[c]["out"] is [512, 1024]: rows 0-255 = batch 0 f-slice
    [256c, 256c+256), rows 256-511 = batch 1 same slice."""
    out_full = np.empty((B, F, HID), dtype=np.float32)
    for c in range(8):
        r = results[c]["out"]
        out_full[0, FS * c : FS * (c + 1), :] = r[0:FS]
        out_full[1, FS * c : FS * (c + 1), :] = r[FS : 2 * FS]
    return out_full


def kernel(query_input, source_input, bias, wq, wk, wv, wo):
    query_input = np.asarray(query_input, dtype=np.float32)
    source_input = np.asarray(source_input, dtype=np.float32)
    bias = np.asarray(bias, dtype=np.float32)
    wq = np.asarray(wq, dtype=np.float32)
    wk = np.asarray(wk, dtype=np.float32)
    wv = np.asarray(wv, dtype=np.float32)
    wo = np.asarray(wo, dtype=np.float32)

    if np.any(bias):
        return _reference_fallback(query_input, source_input, bias, wq, wk, wv, wo)

    in_maps = make_in_maps(query_input, source_input, wq, wk, wv, wo)
    nc = _get_nc()
    res = run_bass_kernel_spmd(nc, in_maps, core_ids=list(range(8)))
    return assemble(res.results)


# revision 26
# speedup vs baseline: 1.3282x; 1.0083x over previous
"""Multi-head attention (B=2, F=T=2048, H=1024, 16 heads x 64) on 8 TRN2
NeuronCores.

v3 design:
  * exp split across engines: per attention step, head j=0's exp runs on
    ACT (exact, free affine scale), head j=1's exp runs on DVE as a
    one-instruction Schraudolph bf16 exp (tensor_scalar mult+add with
    int16 output whose bits ARE the bf16 exp).
  * softmax normalization deferred to AFTER the AllToAll: shards carry 2
    denominator rows; denominators are inverted post-A2A with
    reciprocal_approx_fast on 16 partitions and broadcast via a tiny
    bf16 ones-matmul, folded into the bf16 conversion multiply.
  * AllToAll split per batch; A2A(b0) overlaps b1's attention. Output
    resharding: core c produces f-rows [256c, 256c+256) of BOTH batches.
  * work interleaving: batch-1 projections are emitted as side-closures
    inside batch-0's attention step loop (keeps PE dense/warm and the
    exp engines fed); batch-0's output projection is interleaved into
    batch-1's attention. Only b1's A2A + output projection are exposed.
  * inputs arrive chunk-major ([B*4, 128, HT*512]) so every DMA line is
    8KB contiguous.
"""

from contextlib import ExitStack

import ml_dtypes
import numpy as np

import concourse.bass as bass  # noqa: F401
import concourse.mybir as mybir
import concourse.tile as tile
from concourse import bacc
from concourse.bass_utils import run_bass_kernel_spmd

B, F, T, HID, NH, DH = 2, 2048, 2048, 1024, 16, 64
HT = HID // 128  # 8 h-tiles
TT = T // 128  # 16 key tiles
FC = F // 512  # 4 query chunks per batch
FS = 256  # f-rows per core per batch
SEG = 130 * 256  # A2A shard: 128 A^T rows + 2 denom rows, 256 f-cols
BF16, F32 = mybir.dt.bfloat16, mybir.dt.float32
I16 = mybir.dt.int16
NPBF16 = ml_dtypes.bfloat16

# Schraudolph bf16 exp: bits(int16) = s * ASCHR + BSCHR, s = raw logits.
ASCHR = 16.0 / float(np.log(2.0))  # (128/ln2) * (1/8 logit scale)
BSCHR = 16250.375  # calibrated; rel err +-3.3%, robust to trunc/nearest

OB_CONST = np.tile(np.kron(np.eye(2), np.ones((1, 64))), (8, 1)).astype(NPBF16)

_CACHE: dict = {}


def _build():
    nc = bacc.Bacc("TRN2", target_bir_lowering=False, debug=False, num_devices=8)

    qT = nc.declare_dram_parameter("qT", [B * 4, 128, HT * 512], BF16, isOutput=False)
    sT = nc.declare_dram_parameter("sT", [B * 4, 128, HT * 512], BF16, isOutput=False)
    w3 = nc.declare_dram_parameter("w3", [128, 3072], BF16, isOutput=False)
    wo = nc.declare_dram_parameter("wo", [HID, HID], BF16, isOutput=False)
    ob = nc.declare_dram_parameter("ob", [16, 128], BF16, isOutput=False)
    out = nc.declare_dram_parameter("out", [2 * FS, HID], F32, isOutput=True)

    a2a_in = [nc.dram_tensor(f"a2a_in{b}", [8, SEG], BF16) for b in range(B)]
    a2a_out = [nc.dram_tensor(f"a2a_out{b}", [8, SEG], BF16) for b in range(B)]

    with tile.TileContext(nc) as tc, ExitStack() as ctx:
        persist = ctx.enter_context(tc.tile_pool(name="persist", bufs=1))
        kT_sb = persist.tile([128, B, T], BF16, tag="kT")
        v_sb = persist.tile([128, B, TT, 2, DH + 1], BF16, tag="v")
        qTp_sb = persist.tile([128, B, F], BF16, tag="qTp")
        wo_sb = persist.tile([128, HT, HID], BF16, tag="wo")
        w3_sb = persist.tile([128, 3, HT, 128], BF16, tag="w3")  # k-major
        onesb = persist.tile([16, 128], BF16, tag="onesb")

        nc.vector.memset(v_sb[:, :, :, :, DH : DH + 1], 1.0)
        nc.scalar.dma_start(out=onesb[:, :], in_=ob[:, :])
        for k in (1, 2, 0):  # K first (gates the ramp), then V, then Q
            nc.scalar.dma_start(
                out=w3_sb[:, k, :, :],
                in_=w3[:, 1024 * k : 1024 * (k + 1)].rearrange(
                    "p (a n) -> p a n", a=HT
                ),
            )

        with (
            tc.tile_pool(name="qin", bufs=4) as qin_pool,
            tc.tile_pool(name="sin", bufs=4) as sin_pool,
            tc.tile_pool(name="ptp", bufs=6) as pt_pool,
            tc.tile_pool(name="stg", bufs=3) as stg_pool,
            tc.tile_pool(name="den", bufs=2) as den_pool,
            tc.tile_pool(name="atg", bufs=2) as atg_pool,
            tc.tile_pool(name="stp", bufs=12) as st_pool,
            tc.tile_pool(name="otp", bufs=4) as ot_pool,
            tc.tile_pool(name="s_ps", bufs=2, space="PSUM") as s_ps_pool,
            tc.tile_pool(name="a_ps", bufs=1, space="PSUM") as a_ps_pool,
            tc.tile_pool(name="pj_ps", bufs=2, space="PSUM") as pj_ps,
        ):

            q_t = {0: [None] * 4, 1: [None] * 4}
            s_t = {0: [None] * 4, 1: [None] * 4}

            sc0_halves = []

            def dma_in_s00():
                for h in range(2):
                    sh = sin_pool.tile([128, 4, 512], BF16, tag="sc", name="sc0h")
                    nc.sync.dma_start(
                        out=sh[:, :, :],
                        in_=sT[0, :, 2048 * h : 2048 * (h + 1)].rearrange(
                            "p (a n) -> p a n", a=4
                        ),
                    )
                    sc0_halves.append(sh)

            def dma_in(kind, b, c):
                if kind == "q":
                    qc = qin_pool.tile([128, HT, 512], BF16, tag="qc")
                    nc.sync.dma_start(
                        out=qc[:, :, :],
                        in_=qT[4 * b + c, :, :].rearrange("p (a n) -> p a n", a=HT),
                    )
                    q_t[b][c] = qc
                else:
                    sc = sin_pool.tile([128, HT, 512], BF16, tag="sc")
                    nc.sync.dma_start(
                        out=sc[:, :, :],
                        in_=sT[4 * b + c, :, :].rearrange("p (a n) -> p a n", a=HT),
                    )
                    s_t[b][c] = sc

            def proj_closures(b, qs, ss, use_scalar_copies):
                """Per-chunk projection closures (each emits PE matmuls and
                one PSUM->SBUF copy)."""
                cls = []

                def qk_copy(dst, src):
                    if use_scalar_copies:
                        nc.scalar.copy(out=dst, in_=src)
                    else:
                        nc.vector.tensor_copy(out=dst, in_=src)

                def src_s(c, ht):
                    if b == 0 and c == 0:
                        return sc0_halves[ht // 4][:, ht % 4, :]
                    return ss[c][:, ht, :]

                def mk_k(c):
                    def f():
                        ps = pj_ps.tile([128, 512], F32, tag="pj", name="ps_k")
                        for ht in range(HT):
                            nc.tensor.matmul(
                                ps[:, :],
                                lhsT=w3_sb[:, 1, ht, :],
                                rhs=src_s(c, ht),
                                start=(ht == 0),
                                stop=(ht == HT - 1),
                            )
                        qk_copy(kT_sb[:, b, 512 * c : 512 * (c + 1)], ps[:, :])
                    return f

                def mk_v(c, i):
                    def f():
                        tt = 4 * c + i
                        ps = pj_ps.tile([128, 512], F32, tag="pj", name="ps_v")
                        for ht in range(HT):
                            nc.tensor.matmul(
                                ps[:, 0:128],
                                lhsT=src_s(c, ht)[:, 128 * i : 128 * (i + 1)],
                                rhs=w3_sb[:, 2, ht, :],
                                start=(ht == 0),
                                stop=(ht == HT - 1),
                            )
                        nc.vector.tensor_copy(
                            out=v_sb[:, b, tt, :, 0:DH],
                            in_=ps[:, 0:128].rearrange("p (j d) -> p j d", j=2),
                        )
                    return f

                def mk_q(c):
                    def f():
                        ps = pj_ps.tile([128, 512], F32, tag="pj", name="ps_q")
                        for ht in range(HT):
                            nc.tensor.matmul(
                                ps[:, :],
                                lhsT=w3_sb[:, 0, ht, :],
                                rhs=qs[c][:, ht, :],
                                start=(ht == 0),
                                stop=(ht == HT - 1),
                            )
                        qk_copy(qTp_sb[:, b, 512 * c : 512 * (c + 1)], ps[:, :])
                    return f

                for c in range(4):
                    cls.append(mk_k(c))
                    for i in range(4):
                        cls.append(mk_v(c, i))
                    cls.append(mk_q(c))
                return cls  # 24 closures (6 per chunk), PE-ordered K,V...,Q

            def emit_s_exp(b, fc, tt):
                # per-head S tiles and pt tiles: dependency tracking is
                # tile-granular, so splitting lets exp(j0) start right after
                # S(j0) and P@V(j0) wait only on its own exp engine.
                pts = []
                for j in range(2):
                    sp = s_ps_pool.tile([128, 512], F32, tag=f"s{j}", name="sp")
                    nc.tensor.matmul(
                        sp[:, :],
                        lhsT=kT_sb[
                            64 * j : 64 * (j + 1), b, 128 * tt : 128 * (tt + 1)
                        ],
                        rhs=qTp_sb[
                            64 * j : 64 * (j + 1), b, 512 * fc : 512 * (fc + 1)
                        ],
                        start=True,
                        stop=True,
                        tile_position=(64 * j, 0),
                    )
                    pt = pt_pool.tile([128, 512], BF16, tag=f"pt{j}", name="pt")
                    if j == 0:
                        nc.scalar.activation(
                            out=pt[:, :],
                            in_=sp[:, :],
                            func=mybir.ActivationFunctionType.Exp,
                            scale=float(DH) ** -0.5,
                        )
                    else:
                        with nc.allow_low_precision("schraudolph bf16 exp"):
                            nc.vector.tensor_scalar(
                                out=pt[:, :].bitcast(I16),
                                in0=sp[:, :],
                                scalar1=ASCHR,
                                scalar2=BSCHR,
                                op0=mybir.AluOpType.mult,
                                op1=mybir.AluOpType.add,
                            )
                    pts.append(pt)
                return pts

            def flush(b, fc, a_ps):
                stf = stg_pool.tile([65, 2, 512], BF16, tag="stf")
                with nc.allow_low_precision("bf16 a2a shard"):
                    nc.vector.tensor_copy(
                        out=stf[:, 0, :], in_=a_ps[:, 0, :]
                    )
                    nc.scalar.copy(out=stf[:, 1, :], in_=a_ps[:, 1, :])
                for h in range(2):
                    dst = 2 * fc + h
                    sl = np.s_[256 * h : 256 * (h + 1)]
                    nc.scalar.dma_start(
                        out=a2a_in[b][dst, 0:32768].rearrange(
                            "(j p n) -> p j n", j=2, p=64, n=256
                        ),
                        in_=stf[0:64, :, sl],
                    )
                    nc.scalar.dma_start(
                        out=a2a_in[b][dst, 32768:33280].rearrange(
                            "(o j n) -> o j n", o=1, j=2, n=256
                        ),
                        in_=stf[64:65, :, sl],
                    )

            def outproj_closures(b):
                """Post-A2A(b): reciprocal + broadcast + normalize-multiply +
                output projection, as closures."""
                cls = []
                den2b = den_pool.tile([2, 8, 256], BF16, tag="den2b")
                den2f = den_pool.tile([2, 8, 256], F32, tag="den2f")
                den2r = den_pool.tile([2, 8, 256], F32, tag="den2r")
                den2 = den_pool.tile([2, 8, 256], BF16, tag="den2")
                atg = atg_pool.tile([128, 8, 256], BF16, tag="atg")

                def gather():
                    nc.sync.dma_start(
                        out=den2b[:, :, :],
                        in_=a2a_out[b][:, 32768:33280].rearrange(
                            "a (j n) -> j a n", j=2, n=256
                        ),
                    )
                    nc.scalar.dma_start(
                        out=atg[:, :, :],
                        in_=a2a_out[b][:, 0:32768].rearrange(
                            "a (r n) -> r a n", r=128, n=256
                        ),
                    )

                def recip():
                    nc.vector.tensor_copy(out=den2f[:, :, :], in_=den2b[:, :, :])
                    nc.vector.reciprocal_approx_fast(
                        out=den2r[:, :, :], in_=den2f[:, :, :]
                    )
                    with nc.allow_low_precision("bf16 denom recip"):
                        nc.vector.tensor_copy(out=den2[:, :, :], in_=den2r[:, :, :])

                cls.append(gather)
                cls.append(recip)
                sts = [None] * 8

                def mk_bc(a):
                    def f():
                        bc = pj_ps.tile([128, 512], F32, tag="pj", name="bc")
                        nc.tensor.matmul(
                            bc[:, 0:256],
                            lhsT=onesb[0:2, :],
                            rhs=den2[:, a, :],
                            start=True,
                            stop=True,
                        )
                        st = st_pool.tile([128, 256], BF16, tag="st")
                        with nc.allow_low_precision("bf16 attn normalize"):
                            nc.vector.tensor_mul(
                                out=st[:, :], in0=atg[:, a, :], in1=bc[:, 0:256]
                            )
                        sts[a] = st
                    return f

                for a in range(8):
                    cls.append(mk_bc(a))

                def mk_o(ft, jj):
                    def f():
                        o_ps = pj_ps.tile([128, 512], F32, tag="pj", name="o")
                        for a in range(8):
                            nc.tensor.matmul(
                                o_ps[:, :],
                                lhsT=sts[a][:, 128 * ft : 128 * (ft + 1)],
                                rhs=wo_sb[:, a, 512 * jj : 512 * (jj + 1)],
                                start=(a == 0),
                                stop=(a == 7),
                            )
                        ot = ot_pool.tile([128, 512], F32, tag="ot")
                        if jj == 0:
                            nc.vector.tensor_copy(out=ot[:, :], in_=o_ps[:, :])
                        else:
                            nc.scalar.copy(out=ot[:, :], in_=o_ps[:, :])
                        nc.scalar.dma_start(
                            out=out[
                                256 * b + 128 * ft : 256 * b + 128 * (ft + 1),
                                512 * jj : 512 * (jj + 1),
                            ],
                            in_=ot[:, :],
                        )
                    return f

                for ft in range(2):
                    for jj in range(2):
                        cls.append(mk_o(ft, jj))
                return cls  # 14 closures

            def attention_batch(b, sched):
                """sched: dict step -> list of closures to emit at that step."""
                steps = [(fc, tt) for fc in range(FC) for tt in range(TT)]
                pts = {}
                pts[steps[0]] = emit_s_exp(b, *steps[0])
                a_tiles = {}
                def emit_pv(fc, tt):
                    a_ps = a_tiles[fc]
                    pt = pts.pop((fc, tt))
                    for j in (1, 0):
                        nc.tensor.matmul(
                            a_ps[:, j, :],
                            lhsT=v_sb[:, b, tt, j, :],
                            rhs=pt[j][:, :],
                            start=(tt == 0),
                            stop=(tt == TT - 1),
                        )
                    if tt == TT - 1:
                        flush(b, fc, a_ps)
                        del a_tiles[fc]

                # P@V runs one step behind S/exp: the exp engines get a full
                # extra step of slack, so P@V never waits on them.
                for i, (fc, tt) in enumerate(steps):
                    if tt == 0:
                        a_tiles[fc] = a_ps_pool.tile(
                            [65, 2, 512], F32, tag="a", name="a_acc"
                        )
                    if i + 1 < len(steps):
                        pts[steps[i + 1]] = emit_s_exp(b, *steps[i + 1])
                    for cl in sched.get(i, ()):
                        cl()
                    if i >= 1:
                        emit_pv(*steps[i - 1])
                emit_pv(*steps[-1])
                for i in sorted(k for k in sched if k >= len(steps)):
                    for cl in sched[i]:
                        cl()

            def spread(closures, start, end):
                """Schedule closures evenly over steps [start, end)."""
                sched = {}
                n = len(closures)
                for idx, cl in enumerate(closures):
                    step = start + idx * (end - start) // n
                    sched.setdefault(step, []).append(cl)
                return sched

            # ---- phase structure ------------------------------------------
            # Input DMAs in consumption-priority order (sync queue is FIFO).
            dma_in_s00()
            dma_in("q", 0, 0)
            for c in range(1, 4):
                dma_in("s", 0, c)
            dma_in("q", 0, 1)
            dma_in("s", 1, 0)
            dma_in("q", 0, 2)
            dma_in("s", 1, 1)
            dma_in("q", 0, 3)
            dma_in("s", 1, 2)
            dma_in("s", 1, 3)
            for c in range(4):
                dma_in("q", 1, c)

            p0 = proj_closures(0, q_t[0], s_t[0], use_scalar_copies=False)
            # chunk 0 of batch 0 runs serially (ramp): K, V x4, Q
            for cl in p0[0:6]:
                cl()
            p1 = proj_closures(1, q_t[1], s_t[1], use_scalar_copies=True)

            # b0 attention schedule: K-c by step 4(c-1), V-c(tt) by step tt,
            # Q-c late (needed at step 16c); b1 K/V spread mid, Q last.
            sched0 = {}
            for c in range(1, 4):
                k_cl, v_cls, q_cl = p0[6 * c], p0[6 * c + 1 : 6 * c + 5], p0[6 * c + 5]
                sched0.setdefault(4 * (c - 1), []).append(k_cl)
                for i, cl in enumerate(v_cls):
                    sched0.setdefault(4 * (c - 1) + 1 + (3 * i) // 4, []).append(cl)
                sched0.setdefault(16 * c - 6, []).append(q_cl)
            b1_kv = []
            b1_q = []
            for c in range(4):
                b1_kv.append(p1[6 * c])
                b1_kv.extend(p1[6 * c + 1 : 6 * c + 5])
                b1_q.append(p1[6 * c + 5])
            for step, cls in spread(b1_kv, 16, 56).items():
                sched0.setdefault(step, []).extend(cls)
            for step, cls in spread(b1_q, 56, 64).items():
                sched0.setdefault(step, []).extend(cls)
            attention_batch(0, sched0)

            nc.gpsimd.collective_compute(
                "AllToAll",
                mybir.AluOpType.bypass,
                replica_groups=[[0, 1, 2, 3, 4, 5, 6, 7]],
                ins=[a2a_in[0].ap().opt()],
                outs=[a2a_out[0].ap().opt()],
            )

            # wo only needed from b0's output projection onward.
            nc.scalar.dma_start(
                out=wo_sb[:, :, :],
                in_=wo[:, :].rearrange("(a p) n -> p a n", p=128),
            )

            # b1 attention with b0's output projection interleaved late;
            # b0's final matmul groups run after b1's attention, inside the
            # A2A#2 wait window.
            op0 = outproj_closures(0)
            sched1 = spread(op0[:10], 18, 54)
            sched1.setdefault(58, []).append(op0[10])
            sched1.setdefault(61, []).append(op0[11])
            sched1[64] = op0[12:]
            attention_batch(1, sched1)

            nc.gpsimd.collective_compute(
                "AllToAll",
                mybir.AluOpType.bypass,
                replica_groups=[[0, 1, 2, 3, 4, 5, 6, 7]],
                ins=[a2a_in[1].ap().opt()],
                outs=[a2a_out[1].ap().opt()],
            )

            for cl in outproj_closures(1):
                cl()

    nc.compile()
    return nc


def _get_nc():
    if "nc" not in _CACHE:
        _CACHE["nc"] = _build()
    return _CACHE["nc"]


def _reference_fallback(query_input, source_input, bias, wq, wk, wv, wo):
    """Numpy fallback, only used if bias is unexpectedly nonzero."""
    q = np.einsum("bfh,hnd->bfnd", query_input, wq) * (DH**-0.5)
    k = np.einsum("bth,hnd->btnd", source_input, wk)
    v = np.einsum("bth,hnd->btnd", source_input, wv)
    logits = np.einsum("btnd,bfnd->bnft", k, q) + bias
    logits -= logits.max(axis=-1, keepdims=True)
    w = np.exp(logits)
    w /= w.sum(axis=-1, keepdims=True)
    attn = np.einsum("bnft,btnd->bfnd", w, v)
    return np.einsum("bfnd,ndh->bfh", attn, wo).astype(np.float32)


def _chunk_major(xT):
    """[B, HID, F] -> [B*4, 128, HT*512] with 8KB-contiguous lines:
    out[4b+c, p, a*512+n] = xT[b, a*128+p, 512c+n]."""
    o = np.empty((B * 4, 128, HT * 512), dtype=NPBF16)
    for b in range(B):
        for c in range(4):
            chunk = xT[b, :, 512 * c : 512 * (c + 1)]  # [1024, 512]
            o[4 * b + c] = (
                chunk.reshape(HT, 128, 512).transpose(1, 0, 2).reshape(128, HT * 512)
            )
    return np.ascontiguousarray(o)


def make_in_maps(query_input, source_input, wq, wk, wv, wo):
    wo2 = np.ascontiguousarray(wo.reshape(HID, HID).astype(NPBF16))
    qTb = _chunk_major(np.transpose(query_input, (0, 2, 1)).astype(NPBF16))
    sTb = _chunk_major(np.transpose(source_input, (0, 2, 1)).astype(NPBF16))
    wqh = wq.reshape(HID, NH, DH)
    wkh = wk.reshape(HID, NH, DH)
    wvh = wv.reshape(HID, NH, DH)

    in_maps = []
    for c in range(8):
        sl = np.s_[:, 2 * c : 2 * c + 2, :]
        w3c = np.concatenate(
            [
                wqh[sl].reshape(HID, 128),
                wkh[sl].reshape(HID, 128),
                wvh[sl].reshape(HID, 128),
            ],
            axis=1,
        )  # [1024, 384]
        w3c = (
            w3c.reshape(HT, 128, 3, 128).transpose(1, 2, 0, 3).reshape(128, 3072)
        )
        in_maps.append(
            {
                "qT": qTb,
                "sT": sTb,
                "w3": np.ascontiguousarray(w3c).astype(NPBF16),
                "wo": wo2,
                "ob": OB_CONST,
            }
        )
    return in_maps


def assemble(results):
    """results[c]["out"] is [512, 1024]: rows 0-255 = batch 0 f-slice
    [256c, 256c+256), rows 256-511 = batch 1 same slice."""
    out_full = np.empty((B, F, HID), dtype=np.float32)
    for c in range(8):
        r = results[c]["out"]
        out_full[0, FS * c : FS * (c + 1), :] = r[0:FS]
        out_full[1, FS * c : FS * (c + 1), :] = r[FS : 2 * FS]
    return out_full


def kernel(query_input, source_input, bias, wq, wk, wv, wo):
    query_input = np.asarray(query_input, dtype=np.float32)
    source_input = np.asarray(source_input, dtype=np.float32)
    bias = np.asarray(bias, dtype=np.float32)
    wq = np.asarray(wq, dtype=np.float32)
    wk = np.asarray(wk, dtype=np.float32)
    wv = np.asarray(wv, dtype=np.float32)
    wo = np.asarray(wo, dtype=np.float32)

    if np.any(bias):
        return _reference_fallback(query_input, source_input, bias, wq, wk, wv, wo)

    in_maps = make_in_maps(query_input, source_input, wq, wk, wv, wo)
    nc = _get_nc()
    res = run_bass_kernel_spmd(nc, in_maps, core_ids=list(range(8)))
    return assemble(res.results)
